# revision 18
# baseline (speedup 1.0000x reference)
"""ChannelMamba Trainium2 kernel: 8-core SPMD (4 batches x 2 channel-halves).

Per core: full input projection + channel-conv + x-projections (replicated per
batch pair), then the selective-scan block for its 96-channel half using DVE
tensor_tensor_scan over (d32 x n4)-packed 128-partition tiles, LN + gating, and
the final fwd+reversed-bwd combine. No collectives; host shards/gathers.
"""
import sys
import numpy as np

if '/opt/trn_rl_repo' not in sys.path:
    sys.path.insert(0, '/opt/trn_rl_repo')

L = 1024
D = 192
HN = 96          # channels per core
NST = 16
DTR = 12
NT = 12          # scan tiles per direction (3 d-groups x 4 n-groups)
HC_BF16 = False   # h and C_rep in bf16 for the hC product

_BUILT = {}


def _build():
    import concourse.bass as bass
    from concourse import bacc, mybir
    import concourse.tile as tile

    f32 = mybir.dt.float32
    bf16 = mybir.dt.bfloat16
    AF = mybir.ActivationFunctionType
    OP = mybir.AluOpType

    nc = bacc.Bacc("TRN2", target_bir_lowering=False, debug=False)

    def din(name, shape):
        return nc.dram_tensor(name, list(shape), f32, kind="ExternalInput").ap()

    u0 = din("u0", (128, L)); u1 = din("u1", (64, L))
    WxT0 = din("WxT0", (128, D)); WxT1 = din("WxT1", (64, D))
    WzT0 = din("WzT0", (128, HN)); WzT1 = din("WzT1", (64, HN))
    WxpT0 = din("WxpT0", (128, 44)); WxpT1 = din("WxpT1", (64, 44))
    WxpbT0 = din("WxpbT0", (128, 44)); WxpbT1 = din("WxpbT1", (64, 44))
    WdtT = din("WdtT", (DTR, HN)); WdtbT = din("WdtbT", (DTR, HN))
    cwrows = din("cwrows", (3, L))      # taps
    cbrow = din("cbrow", (1, L))        # conv bias row
    lnrows = din("lnrows", (4, L))      # lnw, lnb, ln1w, ln1b
    scal = din("scal", (HN, 3))         # dt_bias, D, D_b columns
    AflatF = din("AflatF", (128, NT)); AflatB = din("AflatB", (128, NT))
    RG = din("RG", (3 * HN, 128))       # replication lhsT per d-group
    R16J = din("R16J", (4 * NST, 128))  # B/C replication lhsT per n-group
    S4d = nc.dram_tensor("S4", [128, 32], bf16 if HC_BF16 else f32,
                         kind="ExternalInput").ap()
    SH0 = din("SH0", (128, 3 * 128))    # conv shift mats chunk0 [k0|k1|k2]
    SH1 = din("SH1", (64, 3 * 64))      # conv shift mats chunk1
    EA = din("EA", (64, 128))           # cross-chunk k2: P2c1 -> O0 row127
    EB = din("EB", (128, 64))           # cross-chunk k0: P0c0 -> O1 row0
    ONES = din("ONES", (1, 128))
    out_d = nc.dram_tensor("out", [HN, L], f32, kind="ExternalOutput").ap()

    def bcast_row(dram_ap, row, parts):
        t = dram_ap.tensor
        return bass.AP(t, row * L, [[0, parts], [1, L]])

    with tile.TileContext(nc) as tc:
        with (
            tc.tile_pool(name="cst", bufs=1) as cst,
            tc.tile_pool(name="wrk", bufs=1) as wrk,
            tc.tile_pool(name="pipe", bufs=3) as pipe,
            tc.tile_pool(name="psA", bufs=2, space="PSUM") as psA,
            tc.tile_pool(name="psB", bufs=2, space="PSUM") as psB,
            tc.tile_pool(name="psS", bufs=2, space="PSUM") as psS,
            tc.tile_pool(name="psY", bufs=2, space="PSUM") as psY,
        ):
            def load(ap, shape, tag, pool=cst):
                t = pool.tile(list(shape), ap.dtype, tag=tag)
                nc.sync.dma_start(t[:], ap[:])
                return t

            s_u0 = load(u0, (128, L), "u0", pool=wrk); s_u1 = load(u1, (64, L), "u1", pool=wrk)
            s_WxT0 = load(WxT0, (128, D), "WxT0"); s_WxT1 = load(WxT1, (64, D), "WxT1")
            s_WzT0 = load(WzT0, (128, HN), "WzT0"); s_WzT1 = load(WzT1, (64, HN), "WzT1")
            s_Wxp0 = load(WxpT0, (128, 44), "Wxp0"); s_Wxp1 = load(WxpT1, (64, 44), "Wxp1")
            s_Wxpb0 = load(WxpbT0, (128, 44), "Wxpb0"); s_Wxpb1 = load(WxpbT1, (64, 44), "Wxpb1")
            s_WdtT = load(WdtT, (DTR, HN), "WdtT"); s_WdtbT = load(WdtbT, (DTR, HN), "WdtbT")
            s_cb = load(cbrow, (1, L), "cb")
            s_scal = load(scal, (HN, 3), "scal")
            s_AF = load(AflatF, (128, NT), "AF"); s_AB = load(AflatB, (128, NT), "AB")
            s_RG = []
            for g in range(3):
                t = cst.tile([HN, 128], f32, tag=f"RG{g}")
                nc.sync.dma_start(t[:], RG[g * HN:(g + 1) * HN, :])
                s_RG.append(t)
            s_R16J = []
            for j in range(4):
                t = cst.tile([NST, 128], f32, tag=f"R16J{j}")
                nc.sync.dma_start(t[:], R16J[j * NST:(j + 1) * NST, :])
                s_R16J.append(t)
            s_S4 = load(S4d, (128, 32), "S4")
            s_SH0 = load(SH0, (128, 3 * 128), "SH0")
            s_SH1 = load(SH1, (64, 3 * 64), "SH1")
            s_EA = load(EA, (64, 128), "EA"); s_EB = load(EB, (128, 64), "EB")
            s_ONES = load(ONES, (1, 128), "ONES")

            # conv tap broadcasts + ln row broadcasts via DMA (partition step 0)
            s_cwb = []
            for k in range(3):
                t = wrk.tile([128, L], f32, tag=f"cwb{k}", name=f"cwb{k}")
                nc.sync.dma_start(t[:], bcast_row(cwrows, k, 128))
                s_cwb.append(t)


            s_eps = cst.tile([HN, 1], f32, tag="eps")
            nc.vector.memset(s_eps[:], 1e-5)

            MCH = [(0, 128), (128, 64)]   # (offset, rows) chunks of 192

            # ---- x = W_x @ u  (pre-conv), psum -> sbuf ----
            s_x = []
            for ci, (co, cr) in enumerate(MCH):
                xt = wrk.tile([cr, L], f32, tag=f"x{ci}")
                for nh in range(2):
                    ps = psA.tile([cr, 512], f32, tag="psA")
                    nc.tensor.matmul(ps[:], s_WxT0[:, co:co + cr],
                                     s_u0[:, nh * 512:(nh + 1) * 512],
                                     start=True, stop=False)
                    nc.tensor.matmul(ps[:], s_WxT1[:, co:co + cr],
                                     s_u1[:, nh * 512:(nh + 1) * 512],
                                     start=False, stop=True)
                    nc.scalar.copy(xt[:, nh * 512:(nh + 1) * 512], ps[:])
                s_x.append(xt)

            # ---- z half -> silu(z) ----
            s_sz = wrk.tile([HN, L], f32, tag="sz")
            for nh in range(2):
                ps = psA.tile([HN, 512], f32, tag="psA")
                nc.tensor.matmul(ps[:], s_WzT0[:], s_u0[:, nh * 512:(nh + 1) * 512],
                                 start=True, stop=False)
                nc.tensor.matmul(ps[:], s_WzT1[:], s_u1[:, nh * 512:(nh + 1) * 512],
                                 start=False, stop=True)
                nc.scalar.activation(s_sz[:, nh * 512:(nh + 1) * 512], ps[:], AF.Silu)

            # ---- conv: products on DVE/GPS, shifts+bias accumulated on PE ----
            s_P = []
            for k in range(3):
                pk0 = wrk.tile([128, L], f32, tag="u0" if k == 0 else f"P{k}0", name=f"pk0_{k}")
                pk1 = wrk.tile([64, L], f32, tag="u1" if k == 0 else f"P{k}1", name=f"pk1_{k}")
                eng = nc.gpsimd if k < 2 else nc.vector
                eng.tensor_mul(pk0[:], s_x[0][:], s_cwb[k][:])
                eng.tensor_mul(pk1[:], s_x[1][:], s_cwb[k][0:64, :])
                s_P.append((pk0, pk1))

            s_xc = []
            for ci, (co, cr) in enumerate(MCH):
                xct = wrk.tile([cr, L], f32, tag=f"xc{ci}")
                SH = s_SH0 if ci == 0 else s_SH1
                for nh in range(2):
                    sl = slice(nh * 512, (nh + 1) * 512)
                    ps = psA.tile([cr, 512], f32, tag="psA")
                    for k in range(3):
                        nc.tensor.matmul(ps[:], SH[:, k * cr:(k + 1) * cr],
                                         s_P[k][ci][:, sl], start=(k == 0), stop=False)
                    if ci == 0:
                        nc.tensor.matmul(ps[:], s_EA[:], s_P[2][1][:, sl],
                                         start=False, stop=False)
                        nc.tensor.matmul(ps[:], s_ONES[:, 0:cr], s_cb[:, sl],
                                         start=False, stop=True)
                    else:
                        nc.tensor.matmul(ps[:], s_EB[:], s_P[0][0][:, sl],
                                         start=False, stop=False)
                        nc.tensor.matmul(ps[:], s_ONES[:, 0:cr], s_cb[:, sl],
                                         start=False, stop=True)
                    nc.scalar.copy(xct[:, sl], ps[:])
                s_xc.append(xct)

            # ---- xh (this core's half) + reversed copy ----
            s_xh = wrk.tile([HN, L], f32, tag="xh")
            # half selection is baked on host: host always sends the half's
            # rows via scal/WdtT etc.; but xc is full 192 rows. Host passes
            # H via separate weight slices; the half offset is fixed at build
            # time? No: SPMD one program. Use both-half copies:
            # We copy rows [HOFF, HOFF+96) where HOFF is encoded by host via
            # a dedicated selection: simplest = copy both chunks through a
            # PE gather matmul with host matrix HSEL (192x96 -> but K=192>128).
            # Instead: host sends HSEL0 (128,96), HSEL1 (64,96): xh = HSEL0.T@xc0 + HSEL1.T@xc1.
            pass

            s_HSEL0 = load(din("HSEL0", (128, HN)), (128, HN), "HSEL0")
            s_HSEL1 = load(din("HSEL1", (64, HN)), (64, HN), "HSEL1")
            for nh in range(2):
                sl = slice(nh * 512, (nh + 1) * 512)
                ps = psA.tile([HN, 512], f32, tag="psA")
                nc.tensor.matmul(ps[:], s_HSEL0[:], s_xc[0][:, sl], start=True, stop=False)
                nc.tensor.matmul(ps[:], s_HSEL1[:], s_xc[1][:, sl], start=False, stop=True)
                nc.scalar.copy(s_xh[:, sl], ps[:])
            s_xr = wrk.tile([HN, L], f32, tag="xr")
            nc.vector.tensor_copy(s_xr[:], s_xh[:, ::-1])

            # ---- projections (dtr/B/C as separate base-0 psum outs) ----
            def proj(W0, W1, tagp):
                outs = {}
                for name, mo, mr in (("dtr", 0, DTR), ("B", DTR, NST), ("C", DTR + NST, NST)):
                    sb = wrk.tile([mr, L], f32, tag=f"{tagp}{name}")
                    for nh in range(2):
                        sl = slice(nh * 512, (nh + 1) * 512)
                        ps = psA.tile([mr, 512], f32, tag="psA")
                        nc.tensor.matmul(ps[:], W0[:, mo:mo + mr], s_xc[0][:, sl],
                                         start=True, stop=False)
                        nc.tensor.matmul(ps[:], W1[:, mo:mo + mr], s_xc[1][:, sl],
                                         start=False, stop=True)
                        nc.scalar.copy(sb[:, sl], ps[:])
                    outs[name] = sb
                return outs

            prF = proj(s_Wxp0, s_Wxp1, "f")
            prB = proj(s_Wxpb0, s_Wxpb1, "b")

            # ---- dt -> delta = softplus(dt + dt_bias) ----
            def mk_delta(WT, dtr_sb, tag):
                dl = wrk.tile([HN, L], f32, tag=tag)
                for nh in range(2):
                    sl = slice(nh * 512, (nh + 1) * 512)
                    ps = psA.tile([HN, 512], f32, tag="psA")
                    nc.tensor.matmul(ps[:], WT[:], dtr_sb[:, sl], start=True, stop=True)
                    ex = wrk.tile([HN, 512], f32, tag="spex", name=f"ex{tag}{nh}")
                    nc.scalar.activation(ex[:], ps[:], AF.Exp, bias=s_scal[:, 0:1])
                    nc.scalar.activation(dl[:, sl], ex[:], AF.Ln, bias=1.0)
                return dl

            deltaF = mk_delta(s_WdtT, prF["dtr"], "deltaF")
            deltaB = mk_delta(s_WdtbT, prB["dtr"], "deltaB")

            # ---- B/C replication tiles (128 partitions, n4-periodic) ----
            hdt = bf16 if HC_BF16 else f32

            def mk_rep(src, j, dt_, tag, rtag=None):
                shared = tag[:-1].replace("BrF", "Br").replace("BrB", "Br").replace("CrF", "Cr").replace("CrB", "Cr") + tag[-1]
                rep = wrk.tile([128, L], dt_, tag=rtag or shared, name=tag)
                for nh in range(2):
                    sl = slice(nh * 512, (nh + 1) * 512)
                    ps = psB.tile([128, 512], f32, tag="psB")
                    nc.tensor.matmul(ps[:], s_R16J[j][:],
                                     src[:, sl], start=True, stop=True)
                    nc.scalar.copy(rep[:, sl], ps[:])
                return rep

            BrepF = [mk_rep(prF["B"], j, f32, f"BrF{j}") for j in range(4)]  # tag Br{j}
            CrepF = [mk_rep(prF["C"], j, hdt, f"CrF{j}", rtag=f"cwb{j}" if j < 3 else None) for j in range(4)]
            BrepB = [mk_rep(prB["B"], j, f32, f"BrB{j}") for j in range(4)]
            CrepB = [mk_rep(prB["C"], j, hdt, f"CrB{j}", rtag=f"cwb{j}" if j < 3 else None) for j in range(4)]

            # ---- scan block per direction ----
            def scan_dir(delta, Aflat, Brep, Crep, xin, dcol_idx, tagd):
                dx = wrk.tile([HN, L], f32, tag=f"dx{tagd}")
                nc.vector.tensor_mul(dx[:], delta[:], xin[:])
                y_ps = [psY.tile([HN, 512], f32, tag="psY", name=f"yps{tagd}{_h}") for _h in range(2)]
                for t in range(NT):
                    g, j = t // 4, t % 4
                    rg = s_RG[g][:]
                    dA = pipe.tile([128, L], f32, tag="dA", bufs=2)
                    dBu = pipe.tile([128, L], f32, tag="dBu", bufs=1)
                    for nh in range(2):
                        sl = slice(nh * 512, (nh + 1) * 512)
                        psd = psS.tile([128, 512], f32, tag="psS")
                        nc.tensor.matmul(psd[:], rg, delta[:, sl], start=True, stop=True)
                        nc.scalar.activation(dA[:, sl], psd[:], AF.Exp,
                                             scale=Aflat[:, t:t + 1])
                        psx = psS.tile([128, 512], f32, tag="psS")
                        nc.tensor.matmul(psx[:], rg, dx[:, sl], start=True, stop=True)
                        nc.vector.tensor_mul(dBu[:, sl], psx[:], Brep[j][:, sl])
                    h = pipe.tile([128, L], hdt, tag="h", bufs=2)
                    nc.vector.tensor_tensor_scan(h[:], dA[:], dBu[:], 0.0,
                                                 OP.mult, OP.add)
                    hC = pipe.tile([128, L], hdt, tag="hC", bufs=2)
                    eng = nc.gpsimd if (t % 2 == 0) else nc.vector
                    eng.tensor_mul(hC[:], h[:], Crep[j][:])
                    for nh in range(2):
                        sl = slice(nh * 512, (nh + 1) * 512)
                        nc.tensor.matmul(y_ps[nh][g * 32:(g + 1) * 32, :], s_S4[:],
                                         hC[:, sl], start=(j == 0), stop=(j == 3))
                # y = y_psum + x*D ; t = y*sz
                y1 = wrk.tile([HN, L], f32, tag="y1", name=f"y1{tagd}")
                for nh in range(2):
                    sl = slice(nh * 512, (nh + 1) * 512)
                    nc.vector.scalar_tensor_tensor(
                        y1[:, sl], xin[:, sl], s_scal[:, dcol_idx:dcol_idx + 1],
                        y_ps[nh][:], OP.mult, OP.add)
                tt = wrk.tile([HN, L], f32, tag="tt", name=f"tt{tagd}")
                nc.vector.tensor_mul(tt[:], y1[:], s_sz[:])
                return tt

            ttF = scan_dir(deltaF, s_AF, BrepF, CrepF, s_xh, 1, "F")
            ttB = scan_dir(deltaB, s_AB, BrepB, CrepB, s_xr, 2, "B")

            # ---- layernorm + gated combine ----
            def ln_dir(tt, wrow_idx, brow_idx, tagd):
                wrow = wrk.tile([HN, L], f32, tag="lnw", name=f"lnw{tagd}")
                nc.sync.dma_start(wrow[:], bcast_row(lnrows, wrow_idx, HN))
                brow = wrk.tile([HN, L], f32, tag="lnbr", name=f"lnbr{tagd}")
                nc.sync.dma_start(brow[:], bcast_row(lnrows, brow_idx, HN))
                ssum = wrk.tile([HN, 1], f32, tag=f"ss{tagd}")
                nc.vector.tensor_reduce(ssum[:], tt[:], mybir.AxisListType.X, OP.add)
                m = wrk.tile([HN, 1], f32, tag=f"m{tagd}")
                nc.scalar.mul(m[:], ssum[:], -1.0 / L)
                tcen = wrk.tile([HN, L], f32, tag="scrA", name=f"tc{tagd}")
                nc.vector.tensor_scalar_add(tcen[:], tt[:], m[:])
                sq = wrk.tile([HN, L], f32, tag="scrB", name=f"sq{tagd}")
                vs = wrk.tile([HN, 1], f32, tag=f"vs{tagd}")
                nc.scalar.activation(sq[:], tcen[:], AF.Square, accum_out=vs[:])
                sd = wrk.tile([HN, 1], f32, tag=f"sd{tagd}")
                nc.scalar.activation(sd[:], vs[:], AF.Sqrt, bias=s_eps[:],
                                     scale=1.0 / L)
                rstd = wrk.tile([HN, 1], f32, tag=f"rs{tagd}")
                nc.vector.reciprocal(rstd[:], sd[:])
                oc = wrk.tile([HN, L], f32, tag="scrC", name=f"oc{tagd}")
                nc.vector.tensor_scalar_mul(oc[:], tcen[:], rstd[:])
                ws = wrk.tile([HN, L], f32, tag="x0", name=f"ws{tagd}")
                nc.gpsimd.tensor_mul(ws[:], wrow[:], s_sz[:])
                bs = wrk.tile([HN, L], f32, tag="x1", name=f"bs{tagd}")
                nc.gpsimd.tensor_mul(bs[:], brow[:], s_sz[:])
                q = wrk.tile([HN, L], f32, tag="y1", name=f"q{tagd}")
                nc.vector.tensor_mul(q[:], oc[:], ws[:])
                q2 = wrk.tile([HN, L], f32, tag=f"q2{tagd}")
                nc.vector.tensor_add(q2[:], q[:], bs[:])
                return q2

            qF = ln_dir(ttF, 0, 1, "F")
            qB = ln_dir(ttB, 2, 3, "B")
            s_out = wrk.tile([HN, L], f32, tag="scrB", name="outt")
            nc.vector.tensor_add(s_out[:], qF[:], qB[:, ::-1])
            nc.sync.dma_start(out_d[:], s_out[:])

    nc.compile()
    return nc


def _host_inputs(inputs):
    u = np.asarray(inputs['u'], np.float32)
    W_in = np.asarray(inputs['W_in'], np.float32)
    conv_w = np.asarray(inputs['conv_w'], np.float32)
    conv_b = np.asarray(inputs['conv_b'], np.float32)
    W_xproj = np.asarray(inputs['W_xproj'], np.float32)
    W_dtproj = np.asarray(inputs['W_dtproj'], np.float32)
    dt_bias = np.asarray(inputs['dt_bias'], np.float32)
    A_log = np.asarray(inputs['A_log'], np.float32)
    Dp = np.asarray(inputs['D'], np.float32)
    W_xproj_b = np.asarray(inputs['W_xproj_b'], np.float32)
    W_dtproj_b = np.asarray(inputs['W_dtproj_b'], np.float32)
    A_b_log = np.asarray(inputs['A_b_log'], np.float32)
    D_b = np.asarray(inputs['D_b'], np.float32)
    ln = [np.asarray(inputs[k], np.float32) for k in ('ln_w', 'ln_b', 'ln1_w', 'ln1_b')]

    WxT = W_in[:D].T.copy()                       # (c, d)
    WxpT = W_xproj.T.copy(); WxpbT = W_xproj_b.T.copy()
    cw = conv_w[:, 0, 1, :].T.copy()              # (3, L)
    A = -np.exp(A_log); Ab = -np.exp(A_b_log)

    p = np.arange(128)
    RG = np.zeros((3 * HN, 128), np.float32)
    for g in range(3):
        RG[g * HN + 32 * g + p // 4, p] = 1.0
    R16J = np.zeros((4 * NST, 128), np.float32)
    for j in range(4):
        R16J[j * NST + 4 * j + p % 4, p] = 1.0
    S4 = np.zeros((128, 32), np.float32)
    S4[p, p // 4] = 1.0
    if HC_BF16:
        import ml_dtypes
        S4 = S4.astype(ml_dtypes.bfloat16)
    i7 = np.arange(127)
    M0 = np.zeros((128, 128), np.float32); M0[i7, i7 + 1] = 1.0
    M1 = np.eye(128, dtype=np.float32)
    M2 = np.zeros((128, 128), np.float32); M2[i7 + 1, i7] = 1.0
    SH0 = np.concatenate([M0, M1, M2], 1)
    i3 = np.arange(63)
    N0 = np.zeros((64, 64), np.float32); N0[i3, i3 + 1] = 1.0
    N1 = np.eye(64, dtype=np.float32)
    N2 = np.zeros((64, 64), np.float32); N2[i3 + 1, i3] = 1.0
    SH1 = np.concatenate([N0, N1, N2], 1)
    EA = np.zeros((64, 128), np.float32); EA[0, 127] = 1.0
    EB = np.zeros((128, 64), np.float32); EB[127, 0] = 1.0
    ONES = np.ones((1, 128), np.float32)

    in_maps = []
    for core in range(8):
        b, H = core // 2, core % 2
        hs = slice(H * HN, H * HN + HN)
        u_b = u[b].reshape(D, L)
        WzT = W_in[D + H * HN: D + H * HN + HN].T.copy()
        HSEL = np.zeros((D, HN), np.float32)
        HSEL[np.arange(HN) + H * HN, np.arange(HN)] = 1.0
        Aflat = np.zeros((128, NT), np.float32)
        AflatB_ = np.zeros((128, NT), np.float32)
        for t in range(NT):
            g, j = t // 4, t % 4
            Aflat[p, t] = A[hs][32 * g + p // 4, 4 * j + p % 4]
            AflatB_[p, t] = Ab[hs][32 * g + p // 4, 4 * j + p % 4]
        im = dict(
            u0=u_b[:128].copy(), u1=u_b[128:].copy(),
            WxT0=WxT[:128].copy(), WxT1=WxT[128:].copy(),
            WzT0=WzT[:128].copy(), WzT1=WzT[128:].copy(),
            WxpT0=WxpT[:128].copy(), WxpT1=WxpT[128:].copy(),
            WxpbT0=WxpbT[:128].copy(), WxpbT1=WxpbT[128:].copy(),
            WdtT=W_dtproj[hs].T.copy(), WdtbT=W_dtproj_b[hs].T.copy(),
            cwrows=cw, cbrow=conv_b[None, :].copy(),
            lnrows=np.stack(ln, 0),
            scal=np.stack([dt_bias[hs], Dp[hs], D_b[hs]], 1).copy(),
            AflatF=Aflat, AflatB=AflatB_,
            RG=RG, R16J=R16J, S4=S4, SH0=SH0, SH1=SH1, EA=EA, EB=EB,
            ONES=ONES, HSEL0=HSEL[:128].copy(), HSEL1=HSEL[128:].copy(),
        )
        in_maps.append(im)
    return in_maps


def _run(inputs, trace=False):
    from concourse.bass_utils import run_bass_kernel_spmd
    if 'nc' not in _BUILT:
        _BUILT['nc'] = _build()
    in_maps = _host_inputs(inputs)
    res = run_bass_kernel_spmd(_BUILT['nc'], in_maps, list(range(8)), trace=trace)
    out = np.zeros((4, D, L), np.float32)
    for core in range(8):
        b, H = core // 2, core % 2
        out[b, H * HN:(H + 1) * HN, :] = res.results[core]["out"]
    return out.reshape(4, D, 32, 32), res


def kernel(**inputs):
    out, _ = _run(inputs, trace=False)
    return out
